# revision 1
# baseline (speedup 1.0000x reference)
"""Trainium2 Bass kernel for nn_LinearReg_55508157333593.

Computes: loss = (c_omega * 0.001 / N) * sum over all rows/groups of
L2 norms of 25-element groups of weight [100000, 800] f32.

Since each row is 32 contiguous groups of 25 floats and rows are contiguous,
the whole buffer is just 3.2M consecutive 25-float groups. We shard the flat
array across 8 NeuronCores (10M floats each), stream each core's slab through
SBUF as [128, 78125] (each partition owns 3125 consecutive groups), and per
chunk do: ACT square -> DVE per-group reduce -> ACT sqrt (+fused row-sum
accum) -> DVE accumulate. Each core outputs a [128, 1] partial-sum vector;
the host sums 8x128 partials in float64 and applies the scaling.
"""

import sys

import numpy as np

if "/opt/trn_rl_repo" not in sys.path:
    sys.path.insert(0, "/opt/trn_rl_repo")

N_CORES = 8
P = 128                      # SBUF partitions
GROUP = 25                   # elements per group
C_OMEGA = 0.001
N_ROWS = 100000
ROW = 800                    # elements per row
F_PER_PART = (N_ROWS * ROW) // (N_CORES * P)   # 78125 floats/partition/core
CHUNK = 3125                 # floats per partition per compute chunk
N_CHUNKS = F_PER_PART // CHUNK                 # 25
GPC = CHUNK // GROUP         # 125 groups per partition per chunk

_compiled = None
LAST_RESULTS = None          # BassKernelResults of the most recent run


def build(f_per_part=F_PER_PART, chunk=CHUNK, inp_bufs=8):
    """Build and compile the per-core Bass program."""
    import concourse.tile as tile
    from concourse import bacc, mybir

    assert f_per_part % chunk == 0 and chunk % GROUP == 0
    n_chunks = f_per_part // chunk
    gpc = chunk // GROUP
    f32 = mybir.dt.float32

    nc = bacc.Bacc("TRN2", target_bir_lowering=False, debug=False,
                   num_devices=N_CORES)
    x = nc.dram_tensor("x", [P, f_per_part], f32, kind="ExternalInput").ap()
    out = nc.dram_tensor("out", [P, 1], f32, kind="ExternalOutput").ap()

    with tile.TileContext(nc) as tc:
        with (
            tc.tile_pool(name="inp", bufs=inp_bufs) as inp_pool,
            tc.tile_pool(name="sq", bufs=3) as sq_pool,
            tc.tile_pool(name="small", bufs=3) as small_pool,
            tc.tile_pool(name="accp", bufs=1) as acc_pool,
        ):
            acc = acc_pool.tile([P, 1], f32)
            nc.vector.memset(acc[:], 0.0)
            for i in range(n_chunks):
                t = inp_pool.tile([P, chunk], f32)
                nc.sync.dma_start(t[:], x[:, i * chunk:(i + 1) * chunk])
                sq = sq_pool.tile([P, chunk], f32)
                nc.scalar.activation(sq[:], t[:],
                                     mybir.ActivationFunctionType.Square)
                gs = small_pool.tile([P, gpc], f32)
                nc.vector.reduce_sum(
                    gs[:], sq[:].rearrange("p (g k) -> p g k", k=GROUP),
                    axis=mybir.AxisListType.X)
                gn = small_pool.tile([P, gpc], f32)
                pr = small_pool.tile([P, 1], f32)
                nc.scalar.activation(gn[:], gs[:],
                                     mybir.ActivationFunctionType.Sqrt,
                                     accum_out=pr[:])
                nc.vector.tensor_add(acc[:], acc[:], pr[:])
            nc.sync.dma_start(out[:], acc[:])
    nc.compile()
    return nc


def kernel(weight, c_omega):
    global _compiled, LAST_RESULTS
    from concourse.bass_utils import run_bass_kernel_spmd

    if _compiled is None:
        _compiled = build()
    nc = _compiled

    w = np.asarray(weight)
    if w.dtype != np.float32:
        w = w.astype(np.float32)
    w = np.ascontiguousarray(w)
    flat = w.reshape(-1)
    per_core = flat.size // N_CORES
    in_maps = [
        {"x": flat[c * per_core:(c + 1) * per_core].reshape(P, F_PER_PART)}
        for c in range(N_CORES)
    ]
    LAST_RESULTS = run_bass_kernel_spmd(nc, in_maps,
                                        core_ids=list(range(N_CORES)))
    total = 0.0
    for r in LAST_RESULTS.results:
        total += float(r["out"].astype(np.float64).sum())
    loss = total / N_ROWS * (C_OMEGA * float(c_omega))
    return np.float32(loss)


# revision 10
# speedup vs baseline: 1.0067x; 1.0067x over previous
"""Trainium2 Bass kernel for nn_LinearReg_55508157333593.

Computes: loss = (c_omega * 0.001 / N) * sum over all rows/groups of
L2 norms of 25-element groups of weight [100000, 800] f32.

Since each row is 32 contiguous groups of 25 floats and rows are contiguous,
the whole buffer is just 3.2M consecutive 25-float groups. We shard the flat
array across 8 NeuronCores (10M floats each) and stream each core's slab
through SBUF as [128, 78125] (each partition owns 3125 consecutive groups).

Per chunk (raw Bass, manual semaphore pipeline):
  SP:  DMA chunk i into input slot i%B
  ACT: square chunk i (SBUF->SBUF), then sqrt of chunk i-1's group sums
       with a fused per-partition row-sum (accum_out)
  DVE: per-group (25) reduce of squared chunk i, then accumulate chunk
       i-1's row-sum into acc [128, 1]
Each core outputs its [128, 1] partial-sum vector; the host sums 8x128
partials in float64 and applies the scaling.

The chunk schedule ends with a few small chunks so the compute tail after
the last DMA byte is short.
"""

import sys

import numpy as np

if "/opt/trn_rl_repo" not in sys.path:
    sys.path.insert(0, "/opt/trn_rl_repo")

N_CORES = 8
P = 128                      # SBUF partitions
GROUP = 25                   # elements per group
C_OMEGA = 0.001
N_ROWS = 100000
ROW = 800                    # elements per row
F_PER_PART = (N_ROWS * ROW) // (N_CORES * P)   # 78125 floats/partition/core

# chunk schedule (floats per partition; multiples of GROUP, sums to 78125):
# 24 x 3125 for streaming, then 5 x 625 so the post-DMA compute tail is short.
SCHEDULE = [3125] * 24 + [625] * 5

_compiled = None
LAST_RESULTS = None          # BassKernelResults of the most recent run


def build(f_per_part=F_PER_PART, schedule=None, in_bufs=8, sq_bufs=3):
    """Build and compile the per-core raw-Bass program."""
    from concourse import bacc, mybir

    if schedule is None:
        schedule = SCHEDULE
    assert sum(schedule) == f_per_part
    assert all(s % GROUP == 0 for s in schedule)
    n = len(schedule)
    offs = [sum(schedule[:i]) for i in range(n)]
    gpcs = [s // GROUP for s in schedule]
    max_sz = max(schedule)
    max_gpc = max(gpcs)
    f32 = mybir.dt.float32
    Act = mybir.ActivationFunctionType

    nc = bacc.Bacc("TRN2", target_bir_lowering=False, debug=False,
                   num_devices=N_CORES)
    x = nc.dram_tensor("x", [P, f_per_part], f32, kind="ExternalInput").ap()
    out = nc.dram_tensor("out", [P, 1], f32, kind="ExternalOutput").ap()

    B = in_bufs
    S = sq_bufs
    t = [nc.alloc_sbuf_tensor(f"t{b}", [P, max_sz], f32).ap() for b in range(B)]
    sq = [nc.alloc_sbuf_tensor(f"sq{s}", [P, max_sz], f32).ap() for s in range(S)]
    gs = [nc.alloc_sbuf_tensor(f"gs{s}", [P, max_gpc], f32).ap() for s in range(2)]
    gn = [nc.alloc_sbuf_tensor(f"gn{s}", [P, max_gpc], f32).ap() for s in range(2)]
    # per-chunk row sums land in distinct columns; one final reduce -> acc
    pr_wide = nc.alloc_sbuf_tensor("pr_wide", [P, n], f32).ap()
    acc = nc.alloc_sbuf_tensor("acc", [P, 1], f32).ap()

    # One DMA-completion sem per input slot: increments from different
    # in-flight DMAs interleave, so a single shared counting sem cannot
    # identify which chunk landed. Per-slot DMAs are serialized by the
    # slot-reuse handshake, so per-slot counts are unambiguous.
    dma_sems = [nc.alloc_semaphore(f"dma_sem{b}") for b in range(in_bufs)]
    out_sem = nc.alloc_semaphore("out_sem")
    sq_sem = nc.alloc_semaphore("sq_sem")       # ACT square i done
    red_sem = nc.alloc_semaphore("red_sem")     # DVE reduce i done
    sqrt_sem = nc.alloc_semaphore("sqrt_sem")   # ACT sqrt i done
    add_sem = nc.alloc_semaphore("add_sem")     # DVE accumulate i done

    def chunk(ap_list, slot, i):
        return ap_list[slot][:, :schedule[i]]

    with nc.Block() as block:

        @block.sync
        def _(sp):
            for i in range(n):
                if i >= B:
                    # input slot free once its square has been read
                    sp.wait_ge(sq_sem, i - B + 1)
                sp.dma_start(
                    chunk(t, i % B, i), x[:, offs[i]:offs[i] + schedule[i]]
                ).then_inc(dma_sems[i % B], 16)
            sp.wait_ge(add_sem, 1)
            sp.dma_start(out, acc).then_inc(out_sem, 16)
            sp.wait_ge(out_sem, 16)

        @block.scalar
        def _(act):
            def sqrt_of(j):
                act.activation(
                    gn[j % 2][:, :gpcs[j]], gs[j % 2][:, :gpcs[j]],
                    Act.Sqrt, accum_out=pr_wide[:, j:j + 1],
                ).then_inc(sqrt_sem, 1)

            for i in range(n):
                act.wait_ge(dma_sems[i % B], 16 * (i // B + 1))
                if i >= S:
                    # sq slot free once its reduce has been read
                    act.wait_ge(red_sem, i - S + 1)
                act.activation(
                    chunk(sq, i % S, i), chunk(t, i % B, i), Act.Square
                ).then_inc(sq_sem, 1)
                if i >= 1:
                    act.wait_ge(red_sem, i)
                    sqrt_of(i - 1)
            act.wait_ge(red_sem, n)
            sqrt_of(n - 1)

        @block.vector
        def _(dve):
            for i in range(n):
                dve.wait_ge(sq_sem, i + 1)
                if i >= 2:
                    # gs slot free once its sqrt has been read
                    dve.wait_ge(sqrt_sem, i - 1)
                dve.reduce_sum(
                    gs[i % 2][:, :gpcs[i]],
                    chunk(sq, i % S, i).rearrange("p (g k) -> p g k", k=GROUP),
                    axis=mybir.AxisListType.X,
                ).then_inc(red_sem, 1)
            dve.wait_ge(sqrt_sem, n)
            dve.reduce_sum(acc, pr_wide,
                           axis=mybir.AxisListType.X).then_inc(add_sem, 1)

    nc.compile()
    return nc


def kernel(weight, c_omega):
    global _compiled, LAST_RESULTS
    from concourse.bass_utils import run_bass_kernel_spmd

    if _compiled is None:
        _compiled = build()
    nc = _compiled

    w = np.asarray(weight)
    if w.dtype != np.float32:
        w = w.astype(np.float32)
    w = np.ascontiguousarray(w)
    flat = w.reshape(-1)
    per_core = flat.size // N_CORES
    in_maps = [
        {"x": flat[c * per_core:(c + 1) * per_core].reshape(P, F_PER_PART)}
        for c in range(N_CORES)
    ]
    LAST_RESULTS = run_bass_kernel_spmd(nc, in_maps,
                                        core_ids=list(range(N_CORES)))
    total = 0.0
    for r in LAST_RESULTS.results:
        total += float(r["out"].astype(np.float64).sum())
    loss = total / N_ROWS * (C_OMEGA * float(c_omega))
    return np.float32(loss)


# revision 18
# speedup vs baseline: 1.0685x; 1.0614x over previous
"""Trainium2 Bass kernel for nn_LinearReg_55508157333593.

Computes: loss = (c_omega * 0.001 / N) * sum over all rows/groups of
L2 norms of 25-element groups of weight [100000, 800] f32.

Since each row is 32 contiguous groups of 25 floats and rows are contiguous,
the whole buffer is just 3.2M consecutive 25-float groups. We shard the flat
array across 8 NeuronCores (10M floats each) and stream each core's slab
through SBUF as [128, 78125] (each partition owns 3125 consecutive groups).

Per chunk (raw Bass, manual semaphore pipeline):
  SP:  DMA chunk i into input slot i%B
  ACT: square chunk i (SBUF->SBUF), then sqrt of chunk i-1's group sums
       with a fused per-partition row-sum (accum_out)
  DVE: per-group (25) reduce of squared chunk i, then accumulate chunk
       i-1's row-sum into acc [128, 1]
Each core outputs its [128, 1] partial-sum vector; the host sums 8x128
partials in float64 and applies the scaling.

The chunk schedule ends with a few small chunks so the compute tail after
the last DMA byte is short.
"""

import sys

import numpy as np

if "/opt/trn_rl_repo" not in sys.path:
    sys.path.insert(0, "/opt/trn_rl_repo")

N_CORES = 8
P = 128                      # SBUF partitions
GROUP = 25                   # elements per group
C_OMEGA = 0.001
N_ROWS = 100000
ROW = 800                    # elements per row
F_PER_PART = (N_ROWS * ROW) // (N_CORES * P)   # 78125 floats/partition/core

# chunk schedule (floats per partition; multiples of GROUP, sums to 78125):
# 24 x 3125 for streaming, then a descending tail so the serial compute chain
# after the last DMA byte is short.
SCHEDULE = [3125] * 24 + [625] * 4 + [500, 125]

_compiled = None
LAST_RESULTS = None          # BassKernelResults of the most recent run


def build(f_per_part=F_PER_PART, schedule=None, in_bufs=8, sq_bufs=3):
    """Build and compile the per-core raw-Bass program."""
    from concourse import bacc, mybir

    if schedule is None:
        schedule = SCHEDULE
    assert sum(schedule) == f_per_part
    assert all(s % GROUP == 0 for s in schedule)
    n = len(schedule)
    offs = [sum(schedule[:i]) for i in range(n)]
    gpcs = [s // GROUP for s in schedule]
    max_sz = max(schedule)
    max_gpc = max(gpcs)
    f32 = mybir.dt.float32
    Act = mybir.ActivationFunctionType

    nc = bacc.Bacc("TRN2", target_bir_lowering=False, debug=False,
                   num_devices=N_CORES)
    x = nc.dram_tensor("x", [P, f_per_part], f32, kind="ExternalInput").ap()
    # single-partition output: one small DMA descriptor, fast completion
    out = nc.dram_tensor("out", [1, n], f32, kind="ExternalOutput").ap()

    B = in_bufs
    S = sq_bufs
    t = [nc.alloc_sbuf_tensor(f"t{b}", [P, max_sz], f32).ap() for b in range(B)]
    sq = [nc.alloc_sbuf_tensor(f"sq{s}", [P, max_sz], f32).ap() for s in range(S)]
    gs = [nc.alloc_sbuf_tensor(f"gs{s}", [P, max_gpc], f32).ap() for s in range(2)]
    gn = [nc.alloc_sbuf_tensor(f"gn{s}", [P, max_gpc], f32).ap() for s in range(2)]
    # per-chunk row sums land in distinct columns; PE reduces partitions at end
    pr_wide = nc.alloc_sbuf_tensor("pr_wide", [P, n], f32).ap()
    ones = nc.alloc_sbuf_tensor("ones", [P, 1], f32).ap()
    res_sb = nc.alloc_sbuf_tensor("res_sb", [1, n], f32).ap()
    ps = nc.alloc_psum_tensor("ps", [1, n], f32).ap()

    # One DMA-completion sem per input slot: increments from different
    # in-flight DMAs interleave, so a single shared counting sem cannot
    # identify which chunk landed. Per-slot DMAs are serialized by the
    # slot-reuse handshake, so per-slot counts are unambiguous.
    dma_sems = [nc.alloc_semaphore(f"dma_sem{b}") for b in range(in_bufs)]
    out_sem = nc.alloc_semaphore("out_sem")
    sq_sem = nc.alloc_semaphore("sq_sem")       # ACT square i done
    red_sem = nc.alloc_semaphore("red_sem")     # DVE reduce i done
    sqrt_sem = nc.alloc_semaphore("sqrt_sem")   # ACT sqrt i done
    ones_sem = nc.alloc_semaphore("ones_sem")   # ones vector initialized
    mm_sem = nc.alloc_semaphore("mm_sem")       # PE partition-sum done
    cp_sem = nc.alloc_semaphore("cp_sem")       # PSUM->SBUF copy done

    def chunk(ap_list, slot, i):
        return ap_list[slot][:, :schedule[i]]

    with nc.Block(no_gpsimd_drain=True) as block:

        @block.gpsimd
        def _(gp):
            gp.memset(ones, 1.0).then_inc(ones_sem, 1)

        @block.sync
        def _(sp):
            for i in range(n):
                if i >= B:
                    # input slot free once its square has been read
                    sp.wait_ge(sq_sem, i - B + 1)
                sp.dma_start(
                    chunk(t, i % B, i), x[:, offs[i]:offs[i] + schedule[i]]
                ).then_inc(dma_sems[i % B], 16)
            sp.wait_ge(cp_sem, 1)
            sp.dma_start(out, res_sb).then_inc(out_sem, 16)
            sp.wait_ge(out_sem, 16)

        @block.tensor
        def _(pe):
            pe.wait_ge(ones_sem, 1)
            pe.wait_ge(sqrt_sem, n)
            pe.matmul(ps, ones, pr_wide,
                      start=True, stop=True).then_inc(mm_sem, 1)

        @block.scalar
        def _(act):
            def sqrt_of(j):
                act.activation(
                    gn[j % 2][:, :gpcs[j]], gs[j % 2][:, :gpcs[j]],
                    Act.Sqrt, accum_out=pr_wide[:, j:j + 1],
                ).then_inc(sqrt_sem, 1)

            for i in range(n):
                act.wait_ge(dma_sems[i % B], 16 * (i // B + 1))
                if i >= S:
                    # sq slot free once its reduce has been read
                    act.wait_ge(red_sem, i - S + 1)
                act.activation(
                    chunk(sq, i % S, i), chunk(t, i % B, i), Act.Square
                ).then_inc(sq_sem, 1)
                if i >= 1:
                    act.wait_ge(red_sem, i)
                    sqrt_of(i - 1)
            act.wait_ge(red_sem, n)
            sqrt_of(n - 1)

        @block.vector
        def _(dve):
            for i in range(n):
                dve.wait_ge(sq_sem, i + 1)
                if i >= 2:
                    # gs slot free once its sqrt has been read
                    dve.wait_ge(sqrt_sem, i - 1)
                dve.reduce_sum(
                    gs[i % 2][:, :gpcs[i]],
                    chunk(sq, i % S, i).rearrange("p (g k) -> p g k", k=GROUP),
                    axis=mybir.AxisListType.X,
                ).then_inc(red_sem, 1)
            dve.wait_ge(mm_sem, 1)
            dve.tensor_copy(res_sb, ps).then_inc(cp_sem, 1)

    nc.compile()
    return nc


def kernel(weight, c_omega):
    global _compiled, LAST_RESULTS
    from concourse.bass_utils import run_bass_kernel_spmd

    if _compiled is None:
        _compiled = build()
    nc = _compiled

    w = np.asarray(weight)
    if w.dtype != np.float32:
        w = w.astype(np.float32)
    w = np.ascontiguousarray(w)
    flat = w.reshape(-1)
    per_core = flat.size // N_CORES
    in_maps = [
        {"x": flat[c * per_core:(c + 1) * per_core].reshape(P, F_PER_PART)}
        for c in range(N_CORES)
    ]
    LAST_RESULTS = run_bass_kernel_spmd(nc, in_maps,
                                        core_ids=list(range(N_CORES)))
    total = 0.0
    for r in LAST_RESULTS.results:
        total += float(r["out"].astype(np.float64).sum())
    loss = total / N_ROWS * (C_OMEGA * float(c_omega))
    return np.float32(loss)


def selftest_sim(f_per_part=625, schedule=(250, 250, 75, 25, 25),
                 in_bufs=3, sq_bufs=2, seed=0):
    """CoreSim check on a scaled-down instance; returns max rel err."""
    from concourse.bass_interp import CoreSim

    nc = build(f_per_part=f_per_part, schedule=list(schedule),
               in_bufs=in_bufs, sq_bufs=sq_bufs)
    rng = np.random.default_rng(seed)
    xv = rng.standard_normal((P, f_per_part)).astype(np.float32)
    sim = CoreSim(nc)
    sim.tensor("x")[:] = xv
    sim.simulate()
    got = float(np.array(sim.tensor("out")).astype(np.float64).sum())
    g = xv.reshape(P, f_per_part // GROUP, GROUP)
    want = float(np.sqrt((g.astype(np.float64) ** 2).sum(-1)).sum())
    return abs(got - want) / abs(want)
